# revision 2
# baseline (speedup 1.0000x reference)
"""Trainium2 Bass kernel for nn_Blur3d (4x4 separable blur, pad=(2,1)).

Math: reference 2D-convolves each (h, w) = (128, 128) slice with the
normalized 4x4 blur kernel K2 = outer(kc, kr), zero-padded by (2, 1):
    out[i, j] = sum_{bh, bw} K2[bh, bw] * x[i + 1 - bh, j + 1 - bw]
This equals z = Wc^T @ x @ Wr with Wc/Wr banded 128x128 matrices
W[j, i] = k[i + 1 - j] (taps + zero padding encoded in the band).

On-device (per image, x as [h=128 partitions, w=128 free]):
    mm1: u = matmul(lhsT=x, rhs=Wc)  ->  u = x^T Wc   [w part, ho free]
    mm2: z = matmul(lhsT=u, rhs=Wr)  ->  z = Wc^T x Wr [oh part, ow free]
The lhsT.T semantics absorb the inter-pass transpose.

Performance layout: host pre-transposes each core's images to [h, img, w]
and (default) casts to bf16.  Per-partition DMA runs are then chunk*128
elements contiguous (8 KiB bf16) -> full DMA rate, half the bytes of
fp32.  The blur taps [1,3,3,1]/8 are exact in bf16, so mm1/mm2 run as
bf16 matmuls (1 cycle/row vs fp32's 4) accumulating in fp32 PSUM; only
the bf16 rounding of x, u and z contributes error (~2^-9 each, ~2e-3
total vs the 2e-2 gate).  If the taps are NOT exactly representable in
bf16 the kernel falls back to a full-fp32 pipeline (same layout).

The two mandatory PSUM->SBUF copies are split across engines (u on DVE,
z on Activation) so neither exceeds the DMA floor.

Sharding: 4096 independent images, 512 per core, pure data parallel on
the collapsed (batch, c, t) dim across 8 NeuronCores.
"""

import numpy as np

_P = 128          # image height/width and partition count
_NCORES = 8
_CHUNK = 32       # images per DMA chunk
_GROUP = 4        # images per PSUM bank (4 * 128 fp32 = one 2 KiB bank)

# Default build configuration (overridable for experiments via _cfg).
_CFG = dict(mode="bf16", chunk=_CHUNK, group=_GROUP, u_copy="vector",
            z_copy="scalar")

_PROGRAM_CACHE = {}
LAST_RESULTS = None  # BassKernelResults of the most recent run


def _taps_from_kernel2d(k2d):
    """Rank-1 (separable) decomposition of the blur kernel."""
    k2d = np.asarray(k2d, dtype=np.float64)
    U, S, Vt = np.linalg.svd(k2d)
    kc = U[:, 0] * np.sqrt(S[0])
    kr = Vt[0] * np.sqrt(S[0])
    if kc.sum() < 0.0:
        kc, kr = -kc, -kr
    resid = np.abs(np.outer(kc, kr) - k2d).max()
    if resid > 1e-9 * max(1.0, np.abs(k2d).max()):
        raise ValueError(f"blur kernel is not separable (rank-1 resid {resid})")
    return kc, kr


def _band(taps, n=_P):
    """W[j, i] = taps[i + 1 - j]; encodes conv taps + zero padding."""
    taps = np.asarray(taps, dtype=np.float64)
    kh = taps.shape[0]
    W = np.zeros((n, n), dtype=np.float32)
    for b in range(kh):
        off = 1 - b  # input row j = i + 1 - b
        d = np.float32(taps[b])
        idx_i = np.arange(n)
        idx_j = idx_i + off
        m = (idx_j >= 0) & (idx_j < n)
        W[idx_j[m], idx_i[m]] = d
    return W


def _bf16_exact(a):
    import ml_dtypes

    return bool(np.all(a.astype(ml_dtypes.bfloat16).astype(np.float32) == a))


def _rep(it, repeats):
    for _ in range(repeats):
        yield from it


def _build_program(n_imgs, mode="bf16", chunk=_CHUNK, group=_GROUP,
                   repeats=1, u_copy="vector", z_copy="scalar"):
    """Per-core Bass program over pre-transposed [h, img, w] DRAM layout.

    mode: "bf16" (bf16 I/O + bf16 matmuls, fp32 PSUM accumulate) or
          "fp32" (full fp32 pipeline, same layout).
    """
    from contextlib import ExitStack

    import concourse.tile as tile
    from concourse import bacc, mybir

    FP = mybir.dt.float32
    BF = mybir.dt.bfloat16
    DT = BF if mode == "bf16" else FP
    nc = bacc.Bacc("TRN2", target_bir_lowering=False, debug=False)

    x = nc.declare_dram_parameter("x", [_P, n_imgs, _P], DT, isOutput=False)
    w2 = nc.declare_dram_parameter("w2", [_P, 2, _P], DT, isOutput=False)
    out = nc.declare_dram_parameter("out", [_P, n_imgs, _P], DT, isOutput=True)

    assert n_imgs % chunk == 0 and chunk % group == 0

    with tile.TileContext(nc) as tc, ExitStack() as ctx:
        wp = ctx.enter_context(tc.tile_pool(name="w", bufs=1))
        xp = ctx.enter_context(tc.tile_pool(name="x", bufs=3))
        up = ctx.enter_context(tc.tile_pool(name="u", bufs=4))
        op = ctx.enter_context(tc.tile_pool(name="o", bufs=3))
        pu = ctx.enter_context(tc.tile_pool(name="pu", bufs=3, space="PSUM"))
        pz = ctx.enter_context(tc.tile_pool(name="pz", bufs=3, space="PSUM"))
        psc = ctx.enter_context(tc.tile_pool(name="psc", bufs=1, space="PSUM"))

        wt = wp.tile([_P, 2, _P], DT)
        nc.sync.dma_start(wt[:], w2[:])
        wct = wt[:, 0, :]
        wrt = wt[:, 1, :]

        # dummy matmul absorbs the weight-DMA wait on PE
        scr = psc.tile([_P, 1], FP)
        nc.tensor.matmul(scr[:], lhsT=wct, rhs=wt[:, 0, 0:1], start=True,
                         stop=True)

        eng = {"vector": nc.vector.tensor_copy, "scalar": nc.scalar.copy,
               "gpsimd": nc.gpsimd.tensor_copy}
        u_eng = eng[u_copy]
        z_eng = eng[z_copy]

        for c in _rep(range(n_imgs // chunk), repeats):
            xt = xp.tile([_P, chunk, _P], DT)
            nc.sync.dma_start(xt[:], x[:, c * chunk : (c + 1) * chunk, :])
            ot = op.tile([_P, chunk, _P], DT)
            # 1-element DVE touch: absorbs the out-DMA slot-recycle wait so
            # the first z-copy of the chunk carries only its PE wait.
            nc.vector.memset(ot[:, 0, 0:1], 0.0)
            for g in range(chunk // group):
                put = pu.tile([_P, group, _P], FP)
                for j in range(group):
                    i = g * group + j
                    nc.tensor.matmul(put[:, j, :], lhsT=xt[:, i, :], rhs=wct,
                                     start=True, stop=True)
                ust = up.tile([_P, group, _P], DT)
                u_eng(ust[:], put[:])
                pzt = pz.tile([_P, group, _P], FP)
                for j in range(group):
                    nc.tensor.matmul(pzt[:, j, :], lhsT=ust[:, j, :], rhs=wrt,
                                     start=True, stop=True)
                z_eng(ot[:, g * group : (g + 1) * group, :], pzt[:])
            nc.sync.dma_start(out[:, c * chunk : (c + 1) * chunk, :], ot[:])

    nc.compile()
    return nc


def _get_program(n_imgs, cfg):
    key = (n_imgs, cfg["mode"], cfg["chunk"], cfg["group"], cfg["u_copy"],
           cfg["z_copy"])
    if key not in _PROGRAM_CACHE:
        _PROGRAM_CACHE[key] = _build_program(
            n_imgs, mode=cfg["mode"], chunk=cfg["chunk"], group=cfg["group"],
            u_copy=cfg["u_copy"], z_copy=cfg["z_copy"])
    return _PROGRAM_CACHE[key]


def _prep_inputs(imgs, kernel2d, cfg):
    """Host-side prep: band matrices + per-core transposed [h, img, w] x."""
    import ml_dtypes

    n = imgs.shape[0]
    per = n // _NCORES
    kc, kr = _taps_from_kernel2d(kernel2d)
    Wc, Wr = _band(kc), _band(kr)
    if cfg["mode"] == "bf16" and not (_bf16_exact(Wc) and _bf16_exact(Wr)):
        cfg["mode"] = "fp32"  # keep full precision for non-bf16 taps

    dt = ml_dtypes.bfloat16 if cfg["mode"] == "bf16" else np.float32
    # w2[h, 0, :] = Wc[h, :], w2[h, 1, :] = Wr[h, :]
    w2 = np.ascontiguousarray(
        np.stack([Wc, Wr], axis=1).astype(dt))  # [128, 2, 128]
    xs = imgs.astype(dt).reshape(_NCORES, per, _P, _P).transpose(0, 2, 1, 3)
    xs = np.ascontiguousarray(xs)  # [ncores, 128, per, 128]
    return [{"x": xs[i], "w2": w2} for i in range(_NCORES)], per


def kernel(input, kernel, _trace=False, _cfg=None):
    global LAST_RESULTS
    from concourse.bass_utils import run_bass_kernel_spmd

    cfg = dict(_CFG)
    if _cfg:
        cfg.update(_cfg)

    x = np.asarray(input, dtype=np.float32)
    orig_shape = x.shape
    imgs = np.ascontiguousarray(x.reshape(-1, _P, _P))
    n = imgs.shape[0]
    assert n % _NCORES == 0

    in_maps, per = _prep_inputs(imgs, kernel, cfg)
    nc = _get_program(per, cfg)
    res = run_bass_kernel_spmd(
        nc, in_maps, core_ids=list(range(_NCORES)), trace=_trace
    )
    LAST_RESULTS = res
    # out[i] is [128, per, 128] in [h, img, w]; undo transpose + cast.
    outs = np.stack([np.asarray(res.results[i]["out"]) for i in range(_NCORES)])
    full = outs.transpose(0, 2, 1, 3).reshape(n, _P, _P).astype(np.float32)
    return full.reshape(orig_shape)


# revision 10
# speedup vs baseline: 1.0879x; 1.0879x over previous
"""Trainium2 Bass kernel for nn_Blur3d (4x4 separable blur, pad=(2,1)).

Math: reference 2D-convolves each (h, w) = (128, 128) slice with the
normalized 4x4 blur kernel K2 = outer(kc, kr), zero-padded by (2, 1):
    out[i, j] = sum_{bh, bw} K2[bh, bw] * x[i + 1 - bh, j + 1 - bw]
This equals z = Wc^T @ x @ Wr with Wc/Wr banded 128x128 matrices
W[j, i] = k[i + 1 - j] (taps + zero padding encoded in the band).

On-device (per image, x as [h=128 partitions, w=128 free]):
    mm1: u = matmul(lhsT=x, rhs=Wc)  ->  u = x^T Wc   [w part, ho free]
    mm2: z = matmul(lhsT=u, rhs=Wr)  ->  z = Wc^T x Wr [oh part, ow free]
The lhsT.T semantics absorb the inter-pass transpose.

Performance layout: host pre-transposes each core's images to [h, img, w]
and (default) casts to bf16.  Per-partition DMA runs are then chunk*128
elements contiguous (8 KiB bf16) -> full DMA rate, half the bytes of
fp32.  The blur taps [1,3,3,1]/8 are exact in bf16, so mm1/mm2 run as
bf16 matmuls (1 cycle/row vs fp32's 4) accumulating in fp32 PSUM; only
the bf16 rounding of x, u and z contributes error (~2^-9 each, ~2e-3
total vs the 2e-2 gate).  If the taps are NOT exactly representable in
bf16 the kernel falls back to a full-fp32 pipeline (same layout).

The two mandatory PSUM->SBUF copies are split across engines (u on DVE,
z on Activation) so neither exceeds the DMA floor.

Sharding: 4096 independent images, 512 per core, pure data parallel on
the collapsed (batch, c, t) dim across 8 NeuronCores.
"""

import numpy as np

_P = 128          # image height/width and partition count
_NCORES = 8
_CHUNK = 32       # images per DMA chunk
_GROUP = 4        # images per PSUM bank (4 * 128 fp32 = one 2 KiB bank)

# Default build configuration (overridable for experiments via _cfg).
# xbufs/obufs=4: deeper input/output SBUF double-buffering measured ~20%
# faster than 3 on HW (keeps DMA engines fed across chunk boundaries).
_CFG = dict(mode="bf16", chunk=_CHUNK, group=_GROUP, u_copy="vector",
            z_copy="scalar", xbufs=4, ubufs=4, obufs=4, pubufs=3, pzbufs=3)

_PROGRAM_CACHE = {}
LAST_RESULTS = None  # BassKernelResults of the most recent run


def _taps_from_kernel2d(k2d):
    """Rank-1 (separable) decomposition of the blur kernel."""
    k2d = np.asarray(k2d, dtype=np.float64)
    U, S, Vt = np.linalg.svd(k2d)
    kc = U[:, 0] * np.sqrt(S[0])
    kr = Vt[0] * np.sqrt(S[0])
    if kc.sum() < 0.0:
        kc, kr = -kc, -kr
    resid = np.abs(np.outer(kc, kr) - k2d).max()
    if resid > 1e-9 * max(1.0, np.abs(k2d).max()):
        raise ValueError(f"blur kernel is not separable (rank-1 resid {resid})")
    return kc, kr


def _band(taps, n=_P):
    """W[j, i] = taps[i + 1 - j]; encodes conv taps + zero padding."""
    taps = np.asarray(taps, dtype=np.float64)
    kh = taps.shape[0]
    W = np.zeros((n, n), dtype=np.float32)
    for b in range(kh):
        off = 1 - b  # input row j = i + 1 - b
        d = np.float32(taps[b])
        idx_i = np.arange(n)
        idx_j = idx_i + off
        m = (idx_j >= 0) & (idx_j < n)
        W[idx_j[m], idx_i[m]] = d
    return W


def _bf16_exact(a):
    import ml_dtypes

    return bool(np.all(a.astype(ml_dtypes.bfloat16).astype(np.float32) == a))


def _rep(it, repeats):
    for _ in range(repeats):
        yield from it


def _build_program(n_imgs, mode="bf16", chunk=_CHUNK, group=_GROUP,
                   repeats=1, u_copy="vector", z_copy="scalar",
                   xbufs=3, ubufs=4, obufs=3, pubufs=3, pzbufs=3):
    """Per-core Bass program over pre-transposed [h, img, w] DRAM layout.

    mode: "bf16" (bf16 I/O + bf16 matmuls, fp32 PSUM accumulate) or
          "fp32" (full fp32 pipeline, same layout).
    """
    from contextlib import ExitStack

    import concourse.tile as tile
    from concourse import bacc, mybir

    FP = mybir.dt.float32
    BF = mybir.dt.bfloat16
    DT = BF if mode == "bf16" else FP
    nc = bacc.Bacc("TRN2", target_bir_lowering=False, debug=False)

    x = nc.declare_dram_parameter("x", [_P, n_imgs, _P], DT, isOutput=False)
    w2 = nc.declare_dram_parameter("w2", [_P, 2, _P], DT, isOutput=False)
    out = nc.declare_dram_parameter("out", [_P, n_imgs, _P], DT, isOutput=True)

    assert n_imgs % chunk == 0 and chunk % group == 0

    with tile.TileContext(nc) as tc, ExitStack() as ctx:
        wp = ctx.enter_context(tc.tile_pool(name="w", bufs=1))
        xp = ctx.enter_context(tc.tile_pool(name="x", bufs=xbufs))
        up = ctx.enter_context(tc.tile_pool(name="u", bufs=ubufs))
        op = ctx.enter_context(tc.tile_pool(name="o", bufs=obufs))
        banks_per_tile = (group * _P * 4 + 2047) // 2048
        if (pubufs + pzbufs) * banks_per_tile + 1 > 8:
            pubufs = pzbufs = (8 // banks_per_tile - 1) // 2 + 1
            while (pubufs + pzbufs) * banks_per_tile + 1 > 8 and pzbufs > 1:
                pzbufs -= 1
        pu = ctx.enter_context(tc.tile_pool(name="pu", bufs=pubufs,
                                            space="PSUM"))
        pz = ctx.enter_context(tc.tile_pool(name="pz", bufs=pzbufs,
                                            space="PSUM"))
        psc = ctx.enter_context(tc.tile_pool(name="psc", bufs=1, space="PSUM"))

        wt = wp.tile([_P, 2, _P], DT)
        nc.sync.dma_start(wt[:], w2[:])
        wct = wt[:, 0, :]
        wrt = wt[:, 1, :]

        # dummy matmul absorbs the weight-DMA wait on PE
        scr = psc.tile([_P, 1], FP)
        nc.tensor.matmul(scr[:], lhsT=wct, rhs=wt[:, 0, 0:1], start=True,
                         stop=True)

        eng = {"vector": nc.vector.tensor_copy, "scalar": nc.scalar.copy,
               "gpsimd": nc.gpsimd.tensor_copy}
        if u_copy == "alt":
            # balance u/z copies 50/50 across DVE and Activation per group
            rrk = [0]

            def u_eng(dst, src):
                (nc.vector.tensor_copy if rrk[0] % 2 == 0
                 else nc.scalar.copy)(dst, src)
                rrk[0] += 1

            z_eng = u_eng
        else:
            u_eng = eng[u_copy]
            z_eng = eng[z_copy]

        for c in _rep(range(n_imgs // chunk), repeats):
            xt = xp.tile([_P, chunk, _P], DT)
            nc.sync.dma_start(xt[:], x[:, c * chunk : (c + 1) * chunk, :])
            ot = op.tile([_P, chunk, _P], DT)
            # 1-element DVE touch: absorbs the out-DMA slot-recycle wait so
            # the first z-copy of the chunk carries only its PE wait.
            nc.vector.memset(ot[:, 0, 0:1], 0.0)
            for g in range(chunk // group):
                put = pu.tile([_P, group, _P], FP)
                for j in range(group):
                    i = g * group + j
                    nc.tensor.matmul(put[:, j, :], lhsT=xt[:, i, :], rhs=wct,
                                     start=True, stop=True)
                ust = up.tile([_P, group, _P], DT)
                u_eng(ust[:], put[:])
                pzt = pz.tile([_P, group, _P], FP)
                for j in range(group):
                    nc.tensor.matmul(pzt[:, j, :], lhsT=ust[:, j, :], rhs=wrt,
                                     start=True, stop=True)
                z_eng(ot[:, g * group : (g + 1) * group, :], pzt[:])
            nc.sync.dma_start(out[:, c * chunk : (c + 1) * chunk, :], ot[:])

    nc.compile()
    return nc


def _build_kwargs(cfg):
    return dict(mode=cfg["mode"], chunk=cfg["chunk"], group=cfg["group"],
                u_copy=cfg["u_copy"], z_copy=cfg["z_copy"],
                **{k: v for k, v in cfg.items() if k.endswith("bufs")})


def _get_program(n_imgs, cfg):
    key = (n_imgs,) + tuple(sorted(cfg.items()))
    if key not in _PROGRAM_CACHE:
        _PROGRAM_CACHE[key] = _build_program(n_imgs, **_build_kwargs(cfg))
    return _PROGRAM_CACHE[key]


def _prep_inputs(imgs, kernel2d, cfg):
    """Host-side prep: band matrices + per-core transposed [h, img, w] x."""
    import ml_dtypes

    n = imgs.shape[0]
    per = n // _NCORES
    kc, kr = _taps_from_kernel2d(kernel2d)
    Wc, Wr = _band(kc), _band(kr)
    if cfg["mode"] == "bf16" and not (_bf16_exact(Wc) and _bf16_exact(Wr)):
        cfg["mode"] = "fp32"  # keep full precision for non-bf16 taps

    dt = ml_dtypes.bfloat16 if cfg["mode"] == "bf16" else np.float32
    # w2[h, 0, :] = Wc[h, :], w2[h, 1, :] = Wr[h, :]
    w2 = np.ascontiguousarray(
        np.stack([Wc, Wr], axis=1).astype(dt))  # [128, 2, 128]
    xs = imgs.astype(dt).reshape(_NCORES, per, _P, _P).transpose(0, 2, 1, 3)
    xs = np.ascontiguousarray(xs)  # [ncores, 128, per, 128]
    return [{"x": xs[i], "w2": w2} for i in range(_NCORES)], per


def kernel(input, kernel, _trace=False, _cfg=None):
    global LAST_RESULTS
    from concourse.bass_utils import run_bass_kernel_spmd

    cfg = dict(_CFG)
    if _cfg:
        cfg.update(_cfg)

    x = np.asarray(input, dtype=np.float32)
    orig_shape = x.shape
    imgs = np.ascontiguousarray(x.reshape(-1, _P, _P))
    n = imgs.shape[0]
    assert n % _NCORES == 0

    in_maps, per = _prep_inputs(imgs, kernel, cfg)
    nc = _get_program(per, cfg)
    res = run_bass_kernel_spmd(
        nc, in_maps, core_ids=list(range(_NCORES)), trace=_trace
    )
    LAST_RESULTS = res
    # out[i] is [128, per, 128] in [h, img, w]; undo transpose + cast.
    outs = np.stack([np.asarray(res.results[i]["out"]) for i in range(_NCORES)])
    full = outs.transpose(0, 2, 1, 3).reshape(n, _P, _P).astype(np.float32)
    return full.reshape(orig_shape)


# revision 11
# speedup vs baseline: 1.7355x; 1.5953x over previous
"""Trainium2 Bass kernel for nn_Blur3d (4x4 separable blur, pad=(2,1)).

Math: reference 2D-convolves each (h, w) = (128, 128) slice with the
normalized 4x4 blur kernel K2 = outer(kc, kr), zero-padded by (2, 1):
    out[i, j] = sum_{bh, bw} K2[bh, bw] * x[i + 1 - bh, j + 1 - bw]
This equals z = Wc^T @ x @ Wr with Wc/Wr banded 128x128 matrices
W[j, i] = k[i + 1 - j] (taps + zero padding encoded in the band).

On-device (per image, x as [h=128 partitions, w=128 free]):
    mm1: u = matmul(lhsT=x, rhs=Wc)  ->  u = x^T Wc   [w part, ho free]
    mm2: z = matmul(lhsT=u, rhs=Wr)  ->  z = Wc^T x Wr [oh part, ow free]
The lhsT.T semantics absorb the inter-pass transpose.

Performance layout: host pre-transposes each core's images to [h, img, w]
and (default) casts to bf16.  Per-partition DMA runs are then chunk*128
elements contiguous (8 KiB bf16) -> full DMA rate, half the bytes of
fp32.  The blur taps [1,3,3,1]/8 are exact in bf16, so mm1/mm2 run as
bf16 matmuls (1 cycle/row vs fp32's 4) accumulating in fp32 PSUM; only
the bf16 rounding of x, u and z contributes error (~2^-9 each, ~2e-3
total vs the 2e-2 gate).  If the taps are NOT exactly representable in
bf16 the kernel falls back to a full-fp32 pipeline (same layout).

The two mandatory PSUM->SBUF copies are split across engines (u on DVE,
z on Activation) so neither exceeds the DMA floor.

Sharding: 4096 independent images, 512 per core, pure data parallel on
the collapsed (batch, c, t) dim across 8 NeuronCores.
"""

import numpy as np

_P = 128          # image height/width and partition count
_NCORES = 8
_CHUNK = 32       # images per DMA chunk
_GROUP = 4        # images per PSUM bank (4 * 128 fp32 = one 2 KiB bank)

# Default build configuration (overridable for experiments via _cfg).
# xbufs/obufs=4: deeper input/output SBUF double-buffering measured ~20%
# faster than 3 on HW (keeps DMA engines fed across chunk boundaries).
# u_copy="alt": PSUM->SBUF copies alternate DVE/Activation so both the u
# and z copy streams are balanced 50/50 across the two engines.
_CFG = dict(mode="bf16", chunk=_CHUNK, group=_GROUP, u_copy="alt",
            z_copy="scalar", xbufs=4, ubufs=4, obufs=4, pubufs=3, pzbufs=3)

_PROGRAM_CACHE = {}
LAST_RESULTS = None  # BassKernelResults of the most recent run


def _taps_from_kernel2d(k2d):
    """Rank-1 (separable) decomposition of the blur kernel."""
    k2d = np.asarray(k2d, dtype=np.float64)
    U, S, Vt = np.linalg.svd(k2d)
    kc = U[:, 0] * np.sqrt(S[0])
    kr = Vt[0] * np.sqrt(S[0])
    if kc.sum() < 0.0:
        kc, kr = -kc, -kr
    resid = np.abs(np.outer(kc, kr) - k2d).max()
    if resid > 1e-9 * max(1.0, np.abs(k2d).max()):
        raise ValueError(f"blur kernel is not separable (rank-1 resid {resid})")
    return kc, kr


def _band(taps, n=_P):
    """W[j, i] = taps[i + 1 - j]; encodes conv taps + zero padding."""
    taps = np.asarray(taps, dtype=np.float64)
    kh = taps.shape[0]
    W = np.zeros((n, n), dtype=np.float32)
    for b in range(kh):
        off = 1 - b  # input row j = i + 1 - b
        d = np.float32(taps[b])
        idx_i = np.arange(n)
        idx_j = idx_i + off
        m = (idx_j >= 0) & (idx_j < n)
        W[idx_j[m], idx_i[m]] = d
    return W


def _bf16_exact(a):
    import ml_dtypes

    return bool(np.all(a.astype(ml_dtypes.bfloat16).astype(np.float32) == a))


def _rep(it, repeats):
    for _ in range(repeats):
        yield from it


def _build_program(n_imgs, mode="bf16", chunk=_CHUNK, group=_GROUP,
                   repeats=1, u_copy="vector", z_copy="scalar",
                   xbufs=3, ubufs=4, obufs=3, pubufs=3, pzbufs=3):
    """Per-core Bass program over pre-transposed [h, img, w] DRAM layout.

    mode: "bf16" (bf16 I/O + bf16 matmuls, fp32 PSUM accumulate) or
          "fp32" (full fp32 pipeline, same layout).
    """
    from contextlib import ExitStack

    import concourse.tile as tile
    from concourse import bacc, mybir

    FP = mybir.dt.float32
    BF = mybir.dt.bfloat16
    DT = BF if mode == "bf16" else FP
    nc = bacc.Bacc("TRN2", target_bir_lowering=False, debug=False)

    x = nc.declare_dram_parameter("x", [_P, n_imgs, _P], DT, isOutput=False)
    w2 = nc.declare_dram_parameter("w2", [_P, 2, _P], DT, isOutput=False)
    out = nc.declare_dram_parameter("out", [_P, n_imgs, _P], DT, isOutput=True)

    assert n_imgs % chunk == 0 and chunk % group == 0

    with tile.TileContext(nc) as tc, ExitStack() as ctx:
        wp = ctx.enter_context(tc.tile_pool(name="w", bufs=1))
        xp = ctx.enter_context(tc.tile_pool(name="x", bufs=xbufs))
        up = ctx.enter_context(tc.tile_pool(name="u", bufs=ubufs))
        op = ctx.enter_context(tc.tile_pool(name="o", bufs=obufs))
        banks_per_tile = (group * _P * 4 + 2047) // 2048
        if (pubufs + pzbufs) * banks_per_tile + 1 > 8:
            pubufs = pzbufs = (8 // banks_per_tile - 1) // 2 + 1
            while (pubufs + pzbufs) * banks_per_tile + 1 > 8 and pzbufs > 1:
                pzbufs -= 1
        pu = ctx.enter_context(tc.tile_pool(name="pu", bufs=pubufs,
                                            space="PSUM"))
        pz = ctx.enter_context(tc.tile_pool(name="pz", bufs=pzbufs,
                                            space="PSUM"))
        psc = ctx.enter_context(tc.tile_pool(name="psc", bufs=1, space="PSUM"))

        wt = wp.tile([_P, 2, _P], DT)
        nc.sync.dma_start(wt[:], w2[:])
        wct = wt[:, 0, :]
        wrt = wt[:, 1, :]

        # dummy matmul absorbs the weight-DMA wait on PE
        scr = psc.tile([_P, 1], FP)
        nc.tensor.matmul(scr[:], lhsT=wct, rhs=wt[:, 0, 0:1], start=True,
                         stop=True)

        eng = {"vector": nc.vector.tensor_copy, "scalar": nc.scalar.copy,
               "gpsimd": nc.gpsimd.tensor_copy}
        if u_copy == "alt":
            # balance u/z copies 50/50 across DVE and Activation per group
            rrk = [0]

            def u_eng(dst, src):
                (nc.vector.tensor_copy if rrk[0] % 2 == 0
                 else nc.scalar.copy)(dst, src)
                rrk[0] += 1

            z_eng = u_eng
        else:
            u_eng = eng[u_copy]
            z_eng = eng[z_copy]

        for c in _rep(range(n_imgs // chunk), repeats):
            xt = xp.tile([_P, chunk, _P], DT)
            nc.sync.dma_start(xt[:], x[:, c * chunk : (c + 1) * chunk, :])
            ot = op.tile([_P, chunk, _P], DT)
            # 1-element DVE touch: absorbs the out-DMA slot-recycle wait so
            # the first z-copy of the chunk carries only its PE wait.
            nc.vector.memset(ot[:, 0, 0:1], 0.0)
            for g in range(chunk // group):
                put = pu.tile([_P, group, _P], FP)
                for j in range(group):
                    i = g * group + j
                    nc.tensor.matmul(put[:, j, :], lhsT=xt[:, i, :], rhs=wct,
                                     start=True, stop=True)
                ust = up.tile([_P, group, _P], DT)
                u_eng(ust[:], put[:])
                pzt = pz.tile([_P, group, _P], FP)
                for j in range(group):
                    nc.tensor.matmul(pzt[:, j, :], lhsT=ust[:, j, :], rhs=wrt,
                                     start=True, stop=True)
                z_eng(ot[:, g * group : (g + 1) * group, :], pzt[:])
            nc.sync.dma_start(out[:, c * chunk : (c + 1) * chunk, :], ot[:])

    nc.compile()
    return nc


def _build_kwargs(cfg):
    return dict(mode=cfg["mode"], chunk=cfg["chunk"], group=cfg["group"],
                u_copy=cfg["u_copy"], z_copy=cfg["z_copy"],
                **{k: v for k, v in cfg.items() if k.endswith("bufs")})


def _get_program(n_imgs, cfg):
    key = (n_imgs,) + tuple(sorted(cfg.items()))
    if key not in _PROGRAM_CACHE:
        _PROGRAM_CACHE[key] = _build_program(n_imgs, **_build_kwargs(cfg))
    return _PROGRAM_CACHE[key]


def _prep_inputs(imgs, kernel2d, cfg):
    """Host-side prep: band matrices + per-core transposed [h, img, w] x."""
    import ml_dtypes

    n = imgs.shape[0]
    per = n // _NCORES
    kc, kr = _taps_from_kernel2d(kernel2d)
    Wc, Wr = _band(kc), _band(kr)
    if cfg["mode"] == "bf16" and not (_bf16_exact(Wc) and _bf16_exact(Wr)):
        cfg["mode"] = "fp32"  # keep full precision for non-bf16 taps

    dt = ml_dtypes.bfloat16 if cfg["mode"] == "bf16" else np.float32
    # w2[h, 0, :] = Wc[h, :], w2[h, 1, :] = Wr[h, :]
    w2 = np.ascontiguousarray(
        np.stack([Wc, Wr], axis=1).astype(dt))  # [128, 2, 128]
    xs = imgs.astype(dt).reshape(_NCORES, per, _P, _P).transpose(0, 2, 1, 3)
    xs = np.ascontiguousarray(xs)  # [ncores, 128, per, 128]
    return [{"x": xs[i], "w2": w2} for i in range(_NCORES)], per


def kernel(input, kernel, _trace=False, _cfg=None):
    global LAST_RESULTS
    from concourse.bass_utils import run_bass_kernel_spmd

    cfg = dict(_CFG)
    if _cfg:
        cfg.update(_cfg)

    x = np.asarray(input, dtype=np.float32)
    orig_shape = x.shape
    imgs = np.ascontiguousarray(x.reshape(-1, _P, _P))
    n = imgs.shape[0]
    assert n % _NCORES == 0

    in_maps, per = _prep_inputs(imgs, kernel, cfg)
    nc = _get_program(per, cfg)
    res = run_bass_kernel_spmd(
        nc, in_maps, core_ids=list(range(_NCORES)), trace=_trace
    )
    LAST_RESULTS = res
    # out[i] is [128, per, 128] in [h, img, w]; undo transpose + cast.
    outs = np.stack([np.asarray(res.results[i]["out"]) for i in range(_NCORES)])
    full = outs.transpose(0, 2, 1, 3).reshape(n, _P, _P).astype(np.float32)
    return full.reshape(orig_shape)
